# revision 1
# baseline (speedup 1.0000x reference)
"""Trainium2 Bass kernel for nn_AttentionSE3 (graph attention message passing).

Strategy (edge/graph parallel, fully host-prepped ELL layout):
- Attention is a segment softmax over incoming edges of each dst node.  Logits are
  dot(k_edge, q_dst)/sqrt(128) with k,q ~ N(0,1): |logit| <~ 2, so the max-subtraction
  is dropped (softmax is shift-invariant; exp() never overflows here) and
  out[n] = sum_e exp(logit_e) * v_e / sum_e exp(logit_e).
- Host sorts nodes by in-degree, packs them into 128-node blocks, and pads each
  block's per-node edge lists to the block max degree D (degree sorting makes the
  padding ~2%).  Blocks are dealt round-robin to the 8 cores; the per-group capacity
  is the max over the 8 cores so EVERY core runs the same static program (no
  collectives: no node's edges ever span two cores).
- Per (node, d) "slot" the host gathers the edge's key row [128] and value row [96]
  (zero for padding).  A padded slot contributes exactly exp(0)=1 to the softmax
  denominator, so the device subtracts a per-node pad count (exact correction).
  Zero-degree nodes get pad_count = D-1 so the denominator is exactly 1 and the
  output row is 0, matching segment_sum semantics.
- Device program per block: DMA k/v tiles [128 nodes x D*feat]; VectorE multiplies
  k by q (q broadcast over d), reduces dk->logits; ScalarE applies exp (with the
  1/sqrt(128) folded into the activation scale); VectorE reduces d->denominator,
  subtracts pad counts, reciprocates, weights v by exp(logits), reduces d, and
  normalizes.  A tunable share of the two big elementwise multiplies goes to GPSIMD
  to balance engines.  Output accumulates in SBUF and is stored with one DMA.
"""

import numpy as np

import concourse.bacc as bacc
import concourse.mybir as mybir
from concourse import tile
from concourse.bass_utils import run_bass_kernel_spmd

try:
    import ml_dtypes
    BF16_NP = np.dtype(ml_dtypes.bfloat16)
except ImportError:  # pragma: no cover
    BF16_NP = None

N_NODES = 50000
H = 8
P = 128  # nodes per block
N_CORES = 8
SCALE = float(1.0 / np.sqrt(128.0))
F32 = mybir.dt.float32

# Fraction of the d-range of the two big elementwise multiplies routed to GPSIMD
# (engine balancing; VectorE carries the reduces which it alone can do).
GP_FRAC_W1 = 0.70
GP_FRAC_W6 = 0.70

# "f32" or "bf16": dtype of k/v/q inputs and of the two weighting products
# (halves DMA traffic and doubles VectorE elementwise throughput; softmax
# accumulations stay fp32).
DTYPE_MODE = "bf16"
# Replace the dk-reduction (radix-16) with 4 pairwise-halves adds in bf16
# (bf16 tensor_tensor runs 2x; tensor_reduce is stuck at 1x).
TREE_W2 = True
# One pairwise-halves add over d before the weighted-value reduction (rounds
# block capacities up to even, ~+3% traffic; halves the strided 1x reduce).
TREE_W7 = True
# 2 = second halving level (capacities rounded to multiples of 4, ~+8% traffic;
# quarters the strided reduce).
TREE_W7_LEVELS = 1

# value columns permuted from [h(8), cx(12)] to [cx(12), h(8)] so the expw
# broadcast in the weighting multiply lands on a middle AP dim (stride-0 inner
# dims are ~6x slower on VectorE); output is produced in the same [cx, h]
# layout and un-permuted on the host.
PERM_V = np.arange(96).reshape(8, 12).T.reshape(-1)  # new_col cx*8+h -> old h*12+cx
PERM_V_INV = np.argsort(PERM_V)


# ---------------------------------------------------------------- host prep

def prepare(value, key, query0, query1, edge_index, n_nodes=N_NODES, n_cores=N_CORES):
    """Build per-core padded ELL shards.  Returns (in_maps, meta)."""
    value = np.asarray(value, dtype=np.float32)
    key = np.asarray(key, dtype=np.float32)
    query0 = np.asarray(query0, dtype=np.float32)
    query1 = np.asarray(query1, dtype=np.float32)
    n_edges = key.shape[0]

    dst = np.asarray(edge_index[1], dtype=np.int64)
    deg = np.bincount(dst, minlength=n_nodes).astype(np.int64)
    n_pad = -(-n_nodes // (P * n_cores)) * (P * n_cores)  # round up to 1024
    deg_pad = np.concatenate([deg, np.zeros(n_pad - n_nodes, dtype=np.int64)])
    nb = n_pad // P
    ng = nb // n_cores

    order = np.argsort(deg_pad, kind="stable")  # node ids, degree-ascending
    degs_o = deg_pad[order]

    blk_max = degs_o.reshape(nb, P).max(axis=1)
    D_eff = np.maximum(blk_max.reshape(ng, n_cores).max(axis=1), 1).astype(np.int64)
    if TREE_W7:
        m = 4 if TREE_W7_LEVELS >= 2 else 2
        D_eff = (D_eff + m - 1) // m * m  # capacities divisible for halving
    off = np.concatenate([[0], np.cumsum(P * D_eff)]).astype(np.int64)
    S = int(off[-1])  # slots per core

    pos = np.arange(n_pad)
    block = pos // P
    g_of = block // n_cores
    core_of = block % n_cores
    row = pos % P
    Dg = D_eff[g_of]
    base = off[g_of] + row * Dg

    edge_order = np.argsort(dst, kind="stable")
    starts = np.concatenate([[0], np.cumsum(deg)])

    pp = np.repeat(pos, degs_o)
    cum0 = np.concatenate([[0], np.cumsum(degs_o)])[:-1]
    d_idx = np.arange(n_edges) - np.repeat(cum0, degs_o)
    node_of_pp = order[pp]
    edge_ids = edge_order[starts[node_of_pp] + d_idx]
    slot_global = core_of[pp] * S + base[pp] + d_idx

    kp = np.zeros((n_cores * S, 128), dtype=np.float32)
    kp[slot_global] = key[edge_ids]
    vp = np.zeros((n_cores * S, 96), dtype=np.float32)
    vp[slot_global] = value.reshape(n_edges, 96)[:, PERM_V][edge_ids]
    kp = kp.reshape(n_cores, S, 128)
    vp = vp.reshape(n_cores, S, 96)

    qfull = np.concatenate([query0, query1], axis=-1).reshape(n_nodes, 128)
    q_pad = np.zeros((n_pad, 128), dtype=np.float32)
    q_pad[:n_nodes] = qfull
    q_sorted = q_pad[order].reshape(nb, P, 128)

    pc = (Dg - degs_o).astype(np.float32)
    zero_deg = degs_o == 0
    pc[zero_deg] = (Dg[zero_deg] - 1).astype(np.float32)
    pc_sorted = pc.reshape(nb, P)

    dt = BF16_NP if DTYPE_MODE == "bf16" else np.float32
    if DTYPE_MODE == "bf16":
        kp = kp.astype(dt)
        vp = vp.astype(dt)
    in_maps = []
    for c in range(n_cores):
        # pre-tiled layouts: q [128, ng*128], pc [128, ng]
        q_c = np.ascontiguousarray(
            q_sorted[c::n_cores].transpose(1, 0, 2).reshape(P, ng * 128)).astype(dt)
        # pad counts pre-expanded over heads -> the denominator subtract is a
        # plain contiguous tensor_tensor (scalar-AP operands load serially,
        # ~1.6us per op; broadcast APs are worse)
        pc_c = np.repeat(np.ascontiguousarray(pc_sorted[c::n_cores].T), H, axis=1)
        in_maps.append({"kp": kp[c], "vp": vp[c], "q": q_c, "pc": pc_c})

    meta = dict(D_eff=D_eff, off=off, S=S, NG=ng, NB=nb, order=order,
                n_nodes=n_nodes, n_pad=n_pad)
    return in_maps, meta


def unshard_output(out_cores, meta):
    """out_cores: list of [128, NG*96] -> [n_nodes, 32, 3]."""
    ng, nb = meta["NG"], meta["NB"]
    n_cores = len(out_cores)
    order, n_nodes, n_pad = meta["order"], meta["n_nodes"], meta["n_pad"]
    out_sorted = np.zeros((nb, P, 96), dtype=np.float32)
    for c in range(n_cores):
        out_sorted[c::n_cores] = (
            out_cores[c].reshape(P, ng, 96).transpose(1, 0, 2))
    out_sorted = out_sorted.reshape(n_pad, 96)[:, PERM_V_INV]
    out_full = np.zeros((n_nodes, 96), dtype=np.float32)
    mask = order < n_nodes
    out_full[order[mask]] = out_sorted[mask]
    return out_full.reshape(n_nodes, 32, 3)


# ---------------------------------------------------------------- bass kernel

def build(D_eff, S, NG, n_cores=N_CORES):
    D_eff = [int(d) for d in D_eff]
    off = np.concatenate([[0], np.cumsum([P * d for d in D_eff])]).astype(np.int64)

    nc = bacc.Bacc("TRN2", target_bir_lowering=False, debug=False,
                   num_devices=n_cores)
    DT = mybir.dt.bfloat16 if DTYPE_MODE == "bf16" else F32
    kp = nc.declare_dram_parameter("kp", [S, 128], DT, isOutput=False)
    vp = nc.declare_dram_parameter("vp", [S, 96], DT, isOutput=False)
    q = nc.declare_dram_parameter("q", [P, NG * 128], DT, isOutput=False)
    pc = nc.declare_dram_parameter("pc", [P, NG * H], F32, isOutput=False)
    out = nc.declare_dram_parameter("out", [P, NG * 96], F32, isOutput=True)

    mult = mybir.AluOpType.mult
    add = mybir.AluOpType.add
    AX = mybir.AxisListType.X

    with tile.TileContext(nc) as tc:
        with tc.tile_pool(name="res", bufs=1) as res, \
             tc.tile_pool(name="work", bufs=2) as work, \
             tc.tile_pool(name="small", bufs=3) as small:
            q_sb = res.tile([P, NG * 128], DT)
            nc.sync.dma_start(q_sb[:], q[:])
            pc_sb = res.tile([P, NG * H], F32)
            nc.sync.dma_start(pc_sb[:], pc[:])
            out_sb = res.tile([P, NG * 96], F32)
            ss_all = res.tile([P, NG * H], F32)

            for g in range(NG):
                D = D_eff[g]
                s0 = int(off[g])
                kt = work.tile([P, D * 128], DT, tag="kt")
                nc.sync.dma_start(
                    kt[:], kp[s0:s0 + P * D, :].rearrange("(n d) f -> n (d f)", n=P))
                vt = work.tile([P, D * 96], DT, tag="vt")
                nc.sync.dma_start(
                    vt[:], vp[s0:s0 + P * D, :].rearrange("(n d) f -> n (d f)", n=P))

                # w = k * q  (q broadcast over d)   [P, D, H, 16]
                qb = (q_sb[:, g * 128:(g + 1) * 128]
                      .rearrange("n (h k) -> n h k", h=H)
                      .unsqueeze(1).broadcast_to([P, D, H, 16]))
                w = work.tile([P, D * 128], DT, tag="kt")
                w4 = w[:].rearrange("n (d h k) -> n d h k", d=D, h=H)
                k4 = kt[:].rearrange("n (d h k) -> n d h k", d=D, h=H)
                dv = D - int(round(D * GP_FRAC_W1))
                if dv > 0:
                    nc.vector.tensor_tensor(
                        out=w4[:, :dv], in0=k4[:, :dv], in1=qb[:, :dv], op=mult)
                if dv < D:
                    nc.gpsimd.tensor_tensor(
                        out=w4[:, dv:], in0=k4[:, dv:], in1=qb[:, dv:], op=mult)

                # logits (unscaled) = reduce_k w   [P, D*H]
                lg = small.tile([P, D * H], F32, tag="lg")
                if TREE_W2:
                    # radix-16 sum as pairwise halves: bf16 TT runs 2x, reduce 1x
                    t8 = small.tile([P, D * H * 8], DT, tag="t8")
                    nc.vector.tensor_tensor(
                        out=t8[:].rearrange("n (a k) -> n a k", k=8),
                        in0=w[:].rearrange("n (a k) -> n a k", k=16)[:, :, :8],
                        in1=w[:].rearrange("n (a k) -> n a k", k=16)[:, :, 8:],
                        op=add)
                    t4 = small.tile([P, D * H * 4], DT, tag="t4")
                    nc.vector.tensor_tensor(
                        out=t4[:].rearrange("n (a k) -> n a k", k=4),
                        in0=t8[:].rearrange("n (a k) -> n a k", k=8)[:, :, :4],
                        in1=t8[:].rearrange("n (a k) -> n a k", k=8)[:, :, 4:],
                        op=add)
                    t2 = small.tile([P, D * H * 2], DT, tag="t2")
                    nc.vector.tensor_tensor(
                        out=t2[:].rearrange("n (a k) -> n a k", k=2),
                        in0=t4[:].rearrange("n (a k) -> n a k", k=4)[:, :, :2],
                        in1=t4[:].rearrange("n (a k) -> n a k", k=4)[:, :, 2:],
                        op=add)
                    nc.vector.tensor_tensor(
                        out=lg[:],
                        in0=t2[:].rearrange("n (a k) -> n a k", k=2)[:, :, 0],
                        in1=t2[:].rearrange("n (a k) -> n a k", k=2)[:, :, 1],
                        op=add)
                else:
                    nc.vector.tensor_reduce(
                        out=lg[:], in_=w[:].rearrange("n (dh k) -> n dh k", k=16),
                        axis=AX, op=add)

                # expw = exp(scale * logits)
                ew = small.tile([P, D * H], DT, tag="ew")
                nc.scalar.activation(out=ew[:], in_=lg[:],
                                     func=mybir.ActivationFunctionType.Exp,
                                     scale=SCALE)

                # segment sum straight into the resident tile; normalization is
                # deferred to one wide pass after the loop (keeps the tiny
                # subtract/reciprocal off every block's critical chain)
                nc.vector.tensor_reduce(
                    out=ss_all[:, g * H:(g + 1) * H],
                    in_=ew[:].rearrange("n (d h) -> n h d", d=D),
                    axis=AX, op=add)

                # wv = v * expw; v columns are [cx, h] so the expw broadcast is
                # on the middle dim and the inner stays contiguous
                wv = work.tile([P, D * 96], DT, tag="vt")
                wv4 = wv[:].rearrange("n (d c h) -> n d c h", d=D, c=12)
                v4 = vt[:].rearrange("n (d c h) -> n d c h", d=D, c=12)
                eb = (ew[:].rearrange("n (d h) -> n d h", d=D)
                      .unsqueeze(2).broadcast_to([P, D, 12, H]))
                dv6 = D - int(round(D * GP_FRAC_W6))
                if dv6 > 0:
                    nc.vector.tensor_tensor(
                        out=wv4[:, :dv6], in0=v4[:, :dv6], in1=eb[:, :dv6], op=mult)
                if dv6 < D:
                    nc.gpsimd.tensor_tensor(
                        out=wv4[:, dv6:], in0=v4[:, dv6:], in1=eb[:, dv6:], op=mult)

                # unnormalized out = reduce_d wv, straight into out_sb
                og = out_sb[:, g * 96:(g + 1) * 96]
                if TREE_W7:
                    Dh = D // 2
                    th = small.tile([P, Dh * 96], DT, tag="th")
                    wv3 = wv[:].rearrange("n (d ch) -> n d ch", d=D)
                    nc.vector.tensor_tensor(
                        out=th[:].rearrange("n (d ch) -> n d ch", d=Dh),
                        in0=wv3[:, :Dh], in1=wv3[:, Dh:], op=add)
                    red, rd = th, Dh
                    if TREE_W7_LEVELS >= 2:
                        Dq = Dh // 2
                        tq = small.tile([P, Dq * 96], DT, tag="tq")
                        th3 = th[:].rearrange("n (d ch) -> n d ch", d=Dh)
                        nc.vector.tensor_tensor(
                            out=tq[:].rearrange("n (d ch) -> n d ch", d=Dq),
                            in0=th3[:, :Dq], in1=th3[:, Dq:], op=add)
                        red, rd = tq, Dq
                    nc.vector.tensor_reduce(
                        out=og, in_=red[:].rearrange("n (d ch) -> n ch d", d=rd),
                        axis=AX, op=add)
                else:
                    nc.vector.tensor_reduce(
                        out=og, in_=wv[:].rearrange("n (d ch) -> n ch d", d=D),
                        axis=AX, op=add)

            # one wide deferred normalization pass
            dn_all = res.tile([P, NG * H], F32)
            nc.vector.tensor_sub(out=dn_all[:], in0=ss_all[:], in1=pc_sb[:])
            rs_all = res.tile([P, NG * H], F32)
            nc.vector.reciprocal(out=rs_all[:], in_=dn_all[:])
            out2 = res.tile([P, NG * 96], F32)
            nc.vector.tensor_tensor(
                out=out2[:].rearrange("n (g c h) -> n g c h", g=NG, c=12),
                in0=out_sb[:].rearrange("n (g c h) -> n g c h", g=NG, c=12),
                in1=(rs_all[:].rearrange("n (g h) -> n g h", g=NG)
                     .unsqueeze(2).broadcast_to([P, NG, 12, H])),
                op=mult)

            nc.sync.dma_start(out[:], out2[:])

    nc.compile()
    return nc


# ---------------------------------------------------------------- entry point

LAST_RESULT = None  # BassKernelResults of the most recent run (for test harness)


def kernel(value, key, query0, query1, edge_index):
    global LAST_RESULT
    import os
    in_maps, meta = prepare(value, key, query0, query1, edge_index)
    nc = build(meta["D_eff"], meta["S"], meta["NG"])
    res = run_bass_kernel_spmd(nc, in_maps, list(range(N_CORES)),
                               tmpdir=os.environ.get("BASS_SPMD_TMPDIR"))
    LAST_RESULT = res
    out_cores = [res.results[c]["out"] for c in range(N_CORES)]
    return unshard_output(out_cores, meta)



# revision 14
# speedup vs baseline: 1.3341x; 1.3341x over previous
"""Trainium2 Bass kernel for nn_AttentionSE3 (graph attention message passing).

v2 strategy (feature-on-partition transposed ELL layout, TensorE reductions):
- Host sorts nodes by in-degree into 128-node blocks, deals blocks round-robin
  to 8 cores, pads each block's edge lists to the block-group capacity D.
  Per group g the slots form a [D, 128] grid (d-major): col = d*128 + node_row.
- Device tiles are TRANSPOSED: kT [128 feats, D*128], vT [104, D*128] where
  rows 0..95 are value channels (h-major: c = h*12 + cx) and rows 96..103 are
  constant 1.0 (so the weighted-value product's rows 96..103 carry the raw
  exp-weights, giving the softmax denominator for free in the same matmul).
- Per group: DVE computes w = kT * q (q broadcast over d: stride-0 middle dim,
  contiguous 128-wide inner runs -> full 2x bf16 throughput).  TensorE reduces
  over the 16 k-features of each head AND replicates each head's logit to its
  13 output rows in one matmul with a fixed block-ones lhsT [128, 104].
  ScalarE applies exp (scale folded) PSUM->SBUF.  DVE multiplies by vT.
  TensorE then segment-sums over d via D accumulating identity matmuls into a
  [104, 128] PSUM tile (rows 0..95 weighted values, 96..103 denominators).
- Softmax max-subtraction is dropped (logits ~ N(0,1): exp never overflows);
  padded slots contribute exp(0)=1 to the denominator and are corrected by a
  host-computed pad count.  Normalization happens once at the end: denominator
  rows - pad counts, reciprocal, TensorE 8->96 replication, one multiply.
"""

import numpy as np

import concourse.bacc as bacc
import concourse.mybir as mybir
from concourse import tile
from concourse.bass_utils import run_bass_kernel_spmd

try:
    import ml_dtypes
    BF16_NP = np.dtype(ml_dtypes.bfloat16)
except ImportError:  # pragma: no cover
    BF16_NP = None

N_NODES = 50000
H = 8
P = 128  # nodes per block
N_CORES = 8
ROWS = 104  # 96 value channels + 8 ones-rows (denominator trick)
SCALE = float(1.0 / np.sqrt(128.0))
F32 = mybir.dt.float32
BF16 = mybir.dt.bfloat16

# Fraction of the two big elementwise multiplies routed to GPSIMD.
GP_FRAC_K = 0.16
GP_FRAC_V = 0.16
GP_FRAC_T = 0.15  # share of the d-halving tree pass on GPSIMD
# ScalarE exp chunk width (PSUM tile cols; matmuls within are <=512)
EXP_CHUNK = 1024


# ---------------------------------------------------------------- host prep

def prepare(value, key, query0, query1, edge_index, n_nodes=N_NODES, n_cores=N_CORES):
    value = np.asarray(value, dtype=np.float32)
    key = np.asarray(key, dtype=np.float32)
    query0 = np.asarray(query0, dtype=np.float32)
    query1 = np.asarray(query1, dtype=np.float32)
    n_edges = key.shape[0]

    dst = np.asarray(edge_index[1], dtype=np.int64)
    deg = np.bincount(dst, minlength=n_nodes).astype(np.int64)
    n_pad = -(-n_nodes // (P * n_cores)) * (P * n_cores)
    deg_pad = np.concatenate([deg, np.zeros(n_pad - n_nodes, dtype=np.int64)])
    nb = n_pad // P
    ng = nb // n_cores

    order = np.argsort(deg_pad, kind="stable")  # node ids, degree-ascending
    degs_o = deg_pad[order]

    blk_max = degs_o.reshape(nb, P).max(axis=1)
    D_eff = np.maximum(blk_max.reshape(ng, n_cores).max(axis=1), 1).astype(np.int64)
    D_eff = (D_eff + 1) // 2 * 2  # even, for the d-halving tree pass
    off = np.concatenate([[0], np.cumsum(P * D_eff)]).astype(np.int64)
    S = int(off[-1])  # cols per core

    pos = np.arange(n_pad)
    block = pos // P
    g_of = block // n_cores
    core_of = block % n_cores
    row = pos % P

    edge_order = np.argsort(dst, kind="stable")
    starts = np.concatenate([[0], np.cumsum(deg)])

    pp = np.repeat(pos, degs_o)           # padded-node position per real edge
    cum0 = np.concatenate([[0], np.cumsum(degs_o)])[:-1]
    d_idx = np.arange(n_edges) - np.repeat(cum0, degs_o)
    node_of_pp = order[pp]
    edge_ids = edge_order[starts[node_of_pp] + d_idx]
    # d-major slot layout: col = off[g] + d*128 + row
    col_global = core_of[pp] * S + off[g_of[pp]] + d_idx * P + row[pp]

    dt = BF16_NP
    kp_flat = np.zeros((n_cores * S, 128), dtype=dt)
    kp_flat[col_global] = key[edge_ids]
    vp_flat = np.zeros((n_cores * S, 96), dtype=dt)
    vp_flat[col_global] = value.reshape(n_edges, 96)[edge_ids]

    qfull = np.concatenate([query0, query1], axis=-1).reshape(n_nodes, 128)
    q_pad = np.zeros((n_pad, 128), dtype=np.float32)
    q_pad[:n_nodes] = qfull

    pc = (D_eff[g_of] - degs_o[pos]).astype(np.float32)  # pad count per padded node
    zero_deg = degs_o[pos] == 0
    pc[zero_deg] = (D_eff[g_of[zero_deg]] - 1).astype(np.float32)

    ids_blocks = order.reshape(nb, P)

    in_maps = []
    for c in range(n_cores):
        kT = np.ascontiguousarray(kp_flat[c * S:(c + 1) * S].T)  # [128, S]
        vT = np.empty((ROWS, S), dtype=dt)
        vT[:96] = vp_flat[c * S:(c + 1) * S].T
        vT[96:] = np.ones((8, S), dtype=dt)
        ids_c = ids_blocks[c::n_cores]                           # [ng, 128]
        qT = np.ascontiguousarray(
            q_pad[ids_c].transpose(2, 0, 1).reshape(128, ng * P)).astype(dt)
        # pad counts packed [8, ng*P] -> [128, ng*P//16] (partition kp = k*8+p
        # holds wide[p, k*W + j]); rows p identical so only k*W+j matters.
        pc_row = pc.reshape(nb, P)[c::n_cores].reshape(ng * P)
        W = ng * P // 16
        pc_c = np.ascontiguousarray(
            np.broadcast_to(pc_row.reshape(16, 1, W), (16, 8, W))
            .reshape(128, W)).astype(np.float32)
        in_maps.append({"kt": kT, "vt": vT, "qt": qT, "pc": pc_c,
                        "cst": _make_consts()})

    meta = dict(D_eff=D_eff, off=off, S=S, NG=ng, NB=nb, order=order,
                n_nodes=n_nodes, n_pad=n_pad)
    return in_maps, meta


def _make_consts():
    """lhsT constants [128, 304] bf16: block-ones [128,104] | I104 | rep8->96."""
    cst = np.zeros((128, 304), dtype=BF16_NP)
    # ones104: p=(h',k) -> col c: 1 iff h' == head(c); head(c) = c//12 (c<96) else c-96
    pidx = np.arange(128)
    hp = pidx // 16
    for c in range(104):
        hc = c // 12 if c < 96 else c - 96
        cst[hp == hc, c] = 1.0
    # I104 at cols 104:208
    cst[:104, 104:208] = np.eye(104, dtype=np.float32)
    # rep8: rows 0..7 -> 96 cols: 1 iff p == c//12
    for c in range(96):
        cst[c // 12, 208 + c] = 1.0
    return cst


def unshard_output(out_cores, meta):
    """out_cores: list of [96, NG*128] f32 -> [n_nodes, 32, 3]."""
    ng, nb = meta["NG"], meta["NB"]
    n_cores = len(out_cores)
    order, n_nodes, n_pad = meta["order"], meta["n_nodes"], meta["n_pad"]
    out_sorted = np.zeros((nb, P, 96), dtype=np.float32)
    for c in range(n_cores):
        out_sorted[c::n_cores] = (
            out_cores[c].reshape(96, ng, P).transpose(1, 2, 0))
    out_sorted = out_sorted.reshape(n_pad, 96)
    out_full = np.zeros((n_nodes, 96), dtype=np.float32)
    mask = order < n_nodes
    out_full[order[mask]] = out_sorted[mask]
    return out_full.reshape(n_nodes, 32, 3)


# ---------------------------------------------------------------- bass kernel

def build(D_eff, S, NG, n_cores=N_CORES):
    D_eff = [int(d) for d in D_eff]
    off = np.concatenate([[0], np.cumsum([P * d for d in D_eff])]).astype(np.int64)

    nc = bacc.Bacc("TRN2", target_bir_lowering=False, debug=False,
                   num_devices=n_cores)
    kp = nc.declare_dram_parameter("kt", [128, S], BF16, isOutput=False)
    vp = nc.declare_dram_parameter("vt", [ROWS, S], BF16, isOutput=False)
    qp = nc.declare_dram_parameter("qt", [128, NG * P], BF16, isOutput=False)
    pcp = nc.declare_dram_parameter("pc", [128, NG * P // 16], F32, isOutput=False)
    cstp = nc.declare_dram_parameter("cst", [128, 304], BF16, isOutput=False)
    out = nc.declare_dram_parameter("out", [96, NG * P], F32, isOutput=True)

    mult = mybir.AluOpType.mult

    with tile.TileContext(nc) as tc:
        with tc.tile_pool(name="res", bufs=1) as res, \
             tc.tile_pool(name="work", bufs=2) as work, \
             tc.tile_pool(name="stg", bufs=2) as stg, \
             tc.psum_pool(name="pl", bufs=2) as plp, \
             tc.psum_pool(name="acc", bufs=2) as accp, \
             tc.psum_pool(name="rp", bufs=2) as rpp:
            qt_sb = res.tile([128, NG * P], BF16)
            nc.sync.dma_start(qt_sb[:], qp[:])
            cst_sb = res.tile([128, 304], BF16)
            nc.sync.dma_start(cst_sb[:], cstp[:])
            ones104 = cst_sb[:, 0:104]
            I104 = cst_sb[0:104, 104:208]
            rep8 = cst_sb[0:8, 208:304]

            out_sb = res.tile([ROWS, NG * P], F32)

            for g in range(NG):
                D = D_eff[g]
                C = D * P
                s0 = int(off[g])
                kt = work.tile([128, C], BF16, tag="kt")
                nc.sync.dma_start(kt[:], kp[:, s0:s0 + C])
                vt = work.tile([ROWS, C], BF16, tag="vt")
                nc.sync.dma_start(vt[:], vp[:, s0:s0 + C])

                # w = kT * q  (q broadcast over d; contiguous 128-runs)
                w = work.tile([128, C], BF16, tag="w")
                w3 = w[:].rearrange("p (d f) -> p d f", d=D)
                kt3 = kt[:].rearrange("p (d f) -> p d f", d=D)
                qb = (qt_sb[:, g * P:(g + 1) * P]
                      .unsqueeze(1).broadcast_to([128, D, P]))
                dv = D - int(round(D * GP_FRAC_K))
                if dv > 0:
                    nc.vector.tensor_tensor(
                        out=w3[:, :dv], in0=kt3[:, :dv], in1=qb[:, :dv], op=mult)
                if dv < D:
                    nc.gpsimd.tensor_tensor(
                        out=w3[:, dv:], in0=kt3[:, dv:], in1=qb[:, dv:], op=mult)

                # logits (replicated to 104 rows) + exp, chunked through PSUM
                ew = work.tile([ROWS, C], BF16, tag="ew")
                for c0 in range(0, C, EXP_CHUNK):
                    cw = min(EXP_CHUNK, C - c0)
                    pl = plp.tile([ROWS, cw], F32, tag="pl")
                    for m0 in range(0, cw, 512):
                        mw = min(512, cw - m0)
                        nc.tensor.matmul(
                            pl[:, m0:m0 + mw], ones104,
                            w[:, c0 + m0:c0 + m0 + mw],
                            start=True, stop=True)
                    nc.scalar.activation(
                        out=ew[:, c0:c0 + cw], in_=pl[:],
                        func=mybir.ActivationFunctionType.Exp, scale=SCALE)

                # wv = vT * expw   (rows 96..103 = expw: denominator rows)
                wv = work.tile([ROWS, C], BF16, tag="wv")
                cs = P * (D - int(round(D * GP_FRAC_V)))
                if cs > 0:
                    nc.vector.tensor_tensor(
                        out=wv[:, :cs], in0=vt[:, :cs], in1=ew[:, :cs], op=mult)
                if cs < C:
                    nc.gpsimd.tensor_tensor(
                        out=wv[:, cs:], in0=vt[:, cs:], in1=ew[:, cs:], op=mult)

                # one d-halving tree pass (DVE/GPSIMD split), then D/2
                # accumulating identity matmuls for the segment-sum over d
                Dh = D // 2
                Ch = Dh * P
                wvh = work.tile([ROWS, Ch], BF16, tag="wvh")
                ct = P * (Dh - int(round(Dh * GP_FRAC_T)))
                if ct > 0:
                    nc.vector.tensor_tensor(
                        out=wvh[:, :ct], in0=wv[:, :ct],
                        in1=wv[:, Ch:Ch + ct], op=mybir.AluOpType.add)
                if ct < Ch:
                    nc.gpsimd.tensor_tensor(
                        out=wvh[:, ct:], in0=wv[:, ct:Ch],
                        in1=wv[:, Ch + ct:], op=mybir.AluOpType.add)
                acc = accp.tile([ROWS, P], F32, tag="acc")
                for d in range(Dh):
                    nc.tensor.matmul(
                        acc[:], I104, wvh[:, d * P:(d + 1) * P],
                        start=(d == 0), stop=(d == Dh - 1))
                nc.vector.tensor_copy(out_sb[:, g * P:(g + 1) * P], acc[:])

            # ---- endgame: denominators -> reciprocal -> replicate -> scale
            # Reciprocal runs on the denominators packed onto all 128
            # partitions ([8, T] -> [128, T/16] via SBUF->SBUF DMA): the
            # iterative-divide DVE op is ~7 cyc/elem, 16x partition packing
            # makes it cheap.
            T = NG * P
            W = T // 16
            pc_sb = res.tile([128, W], F32)
            nc.sync.dma_start(pc_sb[:], pcp[:])
            dnp = res.tile([128, W], F32)
            for k in range(16):
                nc.sync.dma_start(dnp[k * 8:(k + 1) * 8, :],
                                  out_sb[96:104, k * W:(k + 1) * W])
            sbt = res.tile([128, W], F32)
            nc.vector.tensor_sub(out=sbt[:], in0=dnp[:], in1=pc_sb[:])
            rcp = res.tile([128, W], F32)
            nc.vector.reciprocal(out=rcp[:], in_=sbt[:])
            rcpb = res.tile([128, W], BF16)
            nc.vector.tensor_copy(rcpb[:], rcp[:])
            rcb = res.tile([8, T], BF16)
            for k in range(16):
                nc.sync.dma_start(rcb[:, k * W:(k + 1) * W],
                                  rcpb[k * 8:(k + 1) * 8, :])
            for c0 in range(0, T, 512):
                cw = min(512, T - c0)
                rp = rpp.tile([96, cw], F32, tag="rp")
                nc.tensor.matmul(rp[:], rep8, rcb[:, c0:c0 + cw],
                                 start=True, stop=True)
                st = stg.tile([96, cw], F32, tag="st")
                nc.vector.tensor_tensor(
                    out=st[:], in0=out_sb[0:96, c0:c0 + cw], in1=rp[:], op=mult)
                nc.sync.dma_start(out[:, c0:c0 + cw], st[:])

    nc.compile()
    return nc


# ---------------------------------------------------------------- entry point

LAST_RESULT = None


def kernel(value, key, query0, query1, edge_index):
    global LAST_RESULT
    import os
    in_maps, meta = prepare(value, key, query0, query1, edge_index)
    nc = build(meta["D_eff"], meta["S"], meta["NG"])
    res = run_bass_kernel_spmd(nc, in_maps, list(range(N_CORES)),
                               tmpdir=os.environ.get("BASS_SPMD_TMPDIR"))
    LAST_RESULT = res
    out_cores = [res.results[c]["out"] for c in range(N_CORES)]
    return unshard_output(out_cores, meta)


# revision 16
# speedup vs baseline: 1.7100x; 1.2818x over previous
"""Trainium2 Bass kernel for nn_AttentionSE3 (graph attention message passing).

v2 strategy (feature-on-partition transposed ELL layout, TensorE reductions):
- Host sorts nodes by in-degree into 128-node blocks, deals blocks round-robin
  to 8 cores, pads each block's edge lists to the block-group capacity D.
  Per group g the slots form a [D, 128] grid (d-major): col = d*128 + node_row.
- Device tiles are TRANSPOSED: kT [128 feats, D*128], vT [104, D*128] where
  rows 0..95 are value channels (h-major: c = h*12 + cx) and rows 96..103 are
  constant 1.0 (so the weighted-value product's rows 96..103 carry the raw
  exp-weights, giving the softmax denominator for free in the same matmul).
- Per group: DVE computes w = kT * q (q broadcast over d: stride-0 middle dim,
  contiguous 128-wide inner runs -> full 2x bf16 throughput).  TensorE reduces
  over the 16 k-features of each head AND replicates each head's logit to its
  13 output rows in one matmul with a fixed block-ones lhsT [128, 104].
  ScalarE applies exp (scale folded) PSUM->SBUF.  DVE multiplies by vT.
  TensorE then segment-sums over d via D accumulating identity matmuls into a
  [104, 128] PSUM tile (rows 0..95 weighted values, 96..103 denominators).
- Softmax max-subtraction is dropped (logits ~ N(0,1): exp never overflows);
  padded slots contribute exp(0)=1 to the denominator and are corrected by a
  host-computed pad count.  Normalization happens once at the end: denominator
  rows - pad counts, reciprocal, TensorE 8->96 replication, one multiply.
"""

import numpy as np

import concourse.bacc as bacc
import concourse.mybir as mybir
from concourse import tile
from concourse.bass_utils import run_bass_kernel_spmd

try:
    import ml_dtypes
    BF16_NP = np.dtype(ml_dtypes.bfloat16)
except ImportError:  # pragma: no cover
    BF16_NP = None

N_NODES = 50000
H = 8
P = 128  # nodes per block
N_CORES = 8
ROWS = 104  # 96 value channels + 8 ones-rows (denominator trick)
SCALE = float(1.0 / np.sqrt(128.0))
F32 = mybir.dt.float32
BF16 = mybir.dt.bfloat16

# Fraction of the two big elementwise multiplies routed to GPSIMD.
GP_FRAC_K = 0.0
GP_FRAC_V = 0.0
GP_FRAC_T = 0.0  # share of the d-halving tree pass on GPSIMD
# ScalarE exp chunk width (PSUM tile cols; matmuls within are <=512)
EXP_CHUNK = 1024


# ---------------------------------------------------------------- host prep

def prepare(value, key, query0, query1, edge_index, n_nodes=N_NODES, n_cores=N_CORES):
    value = np.asarray(value, dtype=np.float32)
    key = np.asarray(key, dtype=np.float32)
    query0 = np.asarray(query0, dtype=np.float32)
    query1 = np.asarray(query1, dtype=np.float32)
    n_edges = key.shape[0]

    dst = np.asarray(edge_index[1], dtype=np.int64)
    deg = np.bincount(dst, minlength=n_nodes).astype(np.int64)
    n_pad = -(-n_nodes // (P * n_cores)) * (P * n_cores)
    deg_pad = np.concatenate([deg, np.zeros(n_pad - n_nodes, dtype=np.int64)])
    nb = n_pad // P
    ng = nb // n_cores

    order = np.argsort(deg_pad, kind="stable")  # node ids, degree-ascending
    degs_o = deg_pad[order]

    blk_max = degs_o.reshape(nb, P).max(axis=1)
    D_eff = np.maximum(blk_max.reshape(ng, n_cores).max(axis=1), 1).astype(np.int64)
    D_eff = (D_eff + 1) // 2 * 2  # even, for the d-halving tree pass
    off = np.concatenate([[0], np.cumsum(P * D_eff)]).astype(np.int64)
    S = int(off[-1])  # cols per core

    pos = np.arange(n_pad)
    block = pos // P
    g_of = block // n_cores
    core_of = block % n_cores
    row = pos % P

    edge_order = np.argsort(dst, kind="stable")
    starts = np.concatenate([[0], np.cumsum(deg)])

    pp = np.repeat(pos, degs_o)           # padded-node position per real edge
    cum0 = np.concatenate([[0], np.cumsum(degs_o)])[:-1]
    d_idx = np.arange(n_edges) - np.repeat(cum0, degs_o)
    node_of_pp = order[pp]
    edge_ids = edge_order[starts[node_of_pp] + d_idx]
    # d-major slot layout: col = off[g] + d*128 + row
    col_global = core_of[pp] * S + off[g_of[pp]] + d_idx * P + row[pp]

    dt = BF16_NP
    kp_flat = np.zeros((n_cores * S, 128), dtype=dt)
    kp_flat[col_global] = key[edge_ids]
    vp_flat = np.zeros((n_cores * S, 96), dtype=dt)
    vp_flat[col_global] = value.reshape(n_edges, 96)[edge_ids]

    qfull = np.concatenate([query0, query1], axis=-1).reshape(n_nodes, 128)
    q_pad = np.zeros((n_pad, 128), dtype=np.float32)
    q_pad[:n_nodes] = qfull

    pc = (D_eff[g_of] - degs_o[pos]).astype(np.float32)  # pad count per padded node
    zero_deg = degs_o[pos] == 0
    pc[zero_deg] = (D_eff[g_of[zero_deg]] - 1).astype(np.float32)

    ids_blocks = order.reshape(nb, P)

    in_maps = []
    for c in range(n_cores):
        kT = np.ascontiguousarray(kp_flat[c * S:(c + 1) * S].T)  # [128, S]
        vT = np.empty((ROWS, S), dtype=dt)
        vT[:96] = vp_flat[c * S:(c + 1) * S].T
        vT[96:] = np.ones((8, S), dtype=dt)
        ids_c = ids_blocks[c::n_cores]                           # [ng, 128]
        qT = np.ascontiguousarray(
            q_pad[ids_c].transpose(2, 0, 1).reshape(128, ng * P)).astype(dt)
        # pad counts packed [8, ng*P] -> [128, ng*P//16] (partition kp = k*8+p
        # holds wide[p, k*W + j]); rows p identical so only k*W+j matters.
        pc_row = pc.reshape(nb, P)[c::n_cores].reshape(ng * P)
        W = ng * P // 16
        pc_c = np.ascontiguousarray(
            np.broadcast_to(pc_row.reshape(16, 1, W), (16, 8, W))
            .reshape(128, W)).astype(np.float32)
        in_maps.append({"kt": kT, "vt": vT, "qt": qT, "pc": pc_c,
                        "cst": _make_consts()})

    meta = dict(D_eff=D_eff, off=off, S=S, NG=ng, NB=nb, order=order,
                n_nodes=n_nodes, n_pad=n_pad)
    return in_maps, meta


def _make_consts():
    """lhsT constants [128, 352] bf16: block-ones [128,128] | I128 | rep8->96.
    ones128 cols 104..127 are zero so the padded output rows are exact 0."""
    cst = np.zeros((128, 352), dtype=BF16_NP)
    pidx = np.arange(128)
    hp = pidx // 16
    for c in range(104):
        hc = c // 12 if c < 96 else c - 96
        cst[hp == hc, c] = 1.0
    cst[:, 128:256] = np.eye(128, dtype=np.float32)
    for c in range(96):
        cst[c // 12, 256 + c] = 1.0
    return cst


def unshard_output(out_cores, meta):
    """out_cores: list of [96, NG*128] f32 -> [n_nodes, 32, 3]."""
    ng, nb = meta["NG"], meta["NB"]
    n_cores = len(out_cores)
    order, n_nodes, n_pad = meta["order"], meta["n_nodes"], meta["n_pad"]
    out_sorted = np.zeros((nb, P, 96), dtype=np.float32)
    for c in range(n_cores):
        out_sorted[c::n_cores] = (
            out_cores[c].reshape(96, ng, P).transpose(1, 2, 0))
    out_sorted = out_sorted.reshape(n_pad, 96)
    out_full = np.zeros((n_nodes, 96), dtype=np.float32)
    mask = order < n_nodes
    out_full[order[mask]] = out_sorted[mask]
    return out_full.reshape(n_nodes, 32, 3)


# ---------------------------------------------------------------- bass kernel

def build(D_eff, S, NG, n_cores=N_CORES):
    D_eff = [int(d) for d in D_eff]
    off = np.concatenate([[0], np.cumsum([P * d for d in D_eff])]).astype(np.int64)

    nc = bacc.Bacc("TRN2", target_bir_lowering=False, debug=False,
                   num_devices=n_cores)
    kp = nc.declare_dram_parameter("kt", [128, S], BF16, isOutput=False)
    vp = nc.declare_dram_parameter("vt", [ROWS, S], BF16, isOutput=False)
    qp = nc.declare_dram_parameter("qt", [128, NG * P], BF16, isOutput=False)
    pcp = nc.declare_dram_parameter("pc", [128, NG * P // 16], F32, isOutput=False)
    cstp = nc.declare_dram_parameter("cst", [128, 352], BF16, isOutput=False)
    out = nc.declare_dram_parameter("out", [96, NG * P], F32, isOutput=True)

    mult = mybir.AluOpType.mult

    with tile.TileContext(nc) as tc:
        with tc.tile_pool(name="res", bufs=1) as res, \
             tc.tile_pool(name="kv", bufs=3) as kvp, \
             tc.tile_pool(name="work", bufs=2) as work, \
             tc.tile_pool(name="stg", bufs=2) as stg, \
             tc.psum_pool(name="pl", bufs=2) as plp, \
             tc.psum_pool(name="acc", bufs=2) as accp, \
             tc.psum_pool(name="rp", bufs=2) as rpp:
            qt_sb = res.tile([128, NG * P], BF16)
            nc.sync.dma_start(qt_sb[:], qp[:])
            cst_sb = res.tile([128, 352], BF16)
            nc.sync.dma_start(cst_sb[:], cstp[:])
            ones128 = cst_sb[:, 0:128]
            I128 = cst_sb[:, 128:256]
            rep8 = cst_sb[0:8, 256:352]

            out_sb = res.tile([ROWS, NG * P], F32)

            for g in range(NG):
                D = D_eff[g]
                C = D * P
                s0 = int(off[g])
                kt = kvp.tile([128, C], BF16, tag="kt")
                nc.sync.dma_start(kt[:], kp[:, s0:s0 + C])
                vt = kvp.tile([128, C], BF16, tag="vt")
                nc.sync.dma_start(vt[0:ROWS, :], vp[:, s0:s0 + C])

                # w = kT * q  (q broadcast over d; contiguous 128-runs)
                w = work.tile([128, C], BF16, tag="w")
                w3 = w[:].rearrange("p (d f) -> p d f", d=D)
                kt3 = kt[:].rearrange("p (d f) -> p d f", d=D)
                qb = (qt_sb[:, g * P:(g + 1) * P]
                      .unsqueeze(1).broadcast_to([128, D, P]))
                dv = D - int(round(D * GP_FRAC_K))
                if dv > 0:
                    nc.vector.tensor_tensor(
                        out=w3[:, :dv], in0=kt3[:, :dv], in1=qb[:, :dv], op=mult)
                if dv < D:
                    nc.gpsimd.tensor_tensor(
                        out=w3[:, dv:], in0=kt3[:, dv:], in1=qb[:, dv:], op=mult)

                # logits (replicated to 104 rows) + exp, chunked through PSUM
                ew = work.tile([128, C], BF16, tag="ew")
                for c0 in range(0, C, EXP_CHUNK):
                    cw = min(EXP_CHUNK, C - c0)
                    pl = plp.tile([128, cw], F32, tag="pl")
                    for m0 in range(0, cw, 512):
                        mw = min(512, cw - m0)
                        nc.tensor.matmul(
                            pl[:, m0:m0 + mw], ones128,
                            w[:, c0 + m0:c0 + m0 + mw],
                            start=True, stop=True)
                    nc.scalar.activation(
                        out=ew[:, c0:c0 + cw], in_=pl[:],
                        func=mybir.ActivationFunctionType.Exp, scale=SCALE)

                # wv = vT * expw   (rows 96..103 = expw: denominator rows)
                wv = work.tile([128, C], BF16, tag="wv")
                cs = P * (D - int(round(D * GP_FRAC_V)))
                if cs > 0:
                    nc.vector.tensor_tensor(
                        out=wv[:, :cs], in0=vt[:, :cs], in1=ew[:, :cs], op=mult)
                if cs < C:
                    nc.gpsimd.tensor_tensor(
                        out=wv[:, cs:], in0=vt[:, cs:], in1=ew[:, cs:], op=mult)

                # one d-halving tree pass (DVE/GPSIMD split), then D/2
                # accumulating identity matmuls for the segment-sum over d
                Dh = D // 2
                Ch = Dh * P
                wvh = work.tile([128, Ch], BF16, tag="wvh")
                ct = P * (Dh - int(round(Dh * GP_FRAC_T)))
                if ct > 0:
                    nc.vector.tensor_tensor(
                        out=wvh[:, :ct], in0=wv[:, :ct],
                        in1=wv[:, Ch:Ch + ct], op=mybir.AluOpType.add)
                if ct < Ch:
                    nc.gpsimd.tensor_tensor(
                        out=wvh[:, ct:], in0=wv[:, ct:Ch],
                        in1=wv[:, Ch + ct:], op=mybir.AluOpType.add)
                acc = accp.tile([128, P], F32, tag="acc")
                for d in range(Dh):
                    nc.tensor.matmul(
                        acc[0:ROWS, :], I128[0:ROWS, 0:ROWS],
                        wvh[0:ROWS, d * P:(d + 1) * P],
                        start=(d == 0), stop=(d == Dh - 1))
                nc.vector.tensor_copy(out_sb[:, g * P:(g + 1) * P],
                                      acc[0:ROWS, :])

            # ---- endgame: denominators -> reciprocal -> replicate -> scale
            # Reciprocal runs on the denominators packed onto all 128
            # partitions ([8, T] -> [128, T/16] via SBUF->SBUF DMA): the
            # iterative-divide DVE op is ~7 cyc/elem, 16x partition packing
            # makes it cheap.
            T = NG * P
            W = T // 16
            pc_sb = res.tile([128, W], F32)
            nc.sync.dma_start(pc_sb[:], pcp[:])
            dnp = res.tile([128, W], F32)
            for k in range(16):
                nc.sync.dma_start(dnp[k * 8:(k + 1) * 8, :],
                                  out_sb[96:104, k * W:(k + 1) * W])
            sbt = res.tile([128, W], F32)
            nc.vector.tensor_sub(out=sbt[:], in0=dnp[:], in1=pc_sb[:])
            rcp = res.tile([128, W], F32)
            nc.vector.reciprocal(out=rcp[:], in_=sbt[:])
            rcpb = res.tile([128, W], BF16)
            nc.vector.tensor_copy(rcpb[:], rcp[:])
            rcb = res.tile([8, T], BF16)
            for k in range(16):
                nc.sync.dma_start(rcb[:, k * W:(k + 1) * W],
                                  rcpb[k * 8:(k + 1) * 8, :])
            for c0 in range(0, T, 512):
                cw = min(512, T - c0)
                rp = rpp.tile([96, cw], F32, tag="rp")
                nc.tensor.matmul(rp[:], rep8, rcb[:, c0:c0 + cw],
                                 start=True, stop=True)
                st = stg.tile([96, cw], F32, tag="st")
                nc.vector.tensor_tensor(
                    out=st[:], in0=out_sb[0:96, c0:c0 + cw], in1=rp[:], op=mult)
                nc.sync.dma_start(out[:, c0:c0 + cw], st[:])

    nc.compile()
    return nc


# ---------------------------------------------------------------- entry point

LAST_RESULT = None


def kernel(value, key, query0, query1, edge_index):
    global LAST_RESULT
    import os
    in_maps, meta = prepare(value, key, query0, query1, edge_index)
    nc = build(meta["D_eff"], meta["S"], meta["NG"])
    res = run_bass_kernel_spmd(nc, in_maps, list(range(N_CORES)),
                               tmpdir=os.environ.get("BASS_SPMD_TMPDIR"))
    LAST_RESULT = res
    out_cores = [res.results[c]["out"] for c in range(N_CORES)]
    return unshard_output(out_cores, meta)
